# revision 1
# baseline (speedup 1.0000x reference)
"""Multi-head attention (B=2, T=2048, D=1024, H=16) on 8 TRN2 NeuronCores.

Sharding: 2D (batch x head-group). Core c handles batch b = c // 4 and head
group hg = c % 4 (4 heads = 256 channels of the projected dim). Each core:
  1. Projects its batch's q/k/v against its 256-row weight slices -> QT/KT
     in [j, t] layout and V in [t, j] layout (bf16, fp32 PSUM accumulation).
     V is stored augmented with a ones column per head: [V_h | 1].
     Order Q, V, K so attention never stalls waiting for V.
  2. Per head pair, per 512-wide q tile: S.T = K_h @ Q_h.T (transposed
     scores), U = exp(S.T * scale) (no max subtraction: |S*scale| <= ~16,
     exp fits fp32 easily), then [O.T ; denom] += [V_h | 1].T @ U -- the
     softmax denominator rides the PV matmul for free as output row 64.
     The PV matmuls trail the score/exp stage by one k tile so the PE
     never waits on ScalarE (keeps the HAM clock at 2.4 GHz).
  3. Raw [O.T ; denom] is staged to SBUF; per-block reciprocals run on
     idle DVE cycles; normalization + the output projection for q tile
     qt-1 are woven into the middle of qt's blocks as PE filler.
  4. out_partial.T = woT_chunk.T @ O_norm.T  -> [1024, 2048] fp32.
Host sums the 4 head-group partials per batch, transposes, adds bo.

PSUM discipline: exactly one accumulation group per PSUM bank (hardware
start=True clears has_written bits bank-wide). Engine ops only start at
partition offsets {0, 32, 64, 96}; partition shifts (head m=1 belongs at
rows 64-127 of the stage-E operand but results sit at rows 0-64) use
small SBUF->SBUF DMAs.

All shapes are hardcoded for this problem. kernel() takes the full inputs
and returns the full [2, 2048, 1024] fp32 output.
"""

import numpy as np
import ml_dtypes

import concourse.bass as bass
import concourse.bacc as bacc
import concourse.mybir as mybir
import concourse.tile as tile
from concourse.bass_utils import run_bass_kernel_spmd

B, T, D, H, Hd = 2, 2048, 1024, 16, 64
HPC = 4          # heads per core
W = HPC * Hd     # 256 projected channels per core
SCALE = Hd ** -0.5
N_CORES = 8

BF16 = mybir.dt.bfloat16
F32 = mybir.dt.float32
bf16 = ml_dtypes.bfloat16


def build_nc():
    nc = bacc.Bacc("TRN2", target_bir_lowering=False, debug=False)

    xq = nc.dram_tensor("xq", [D, T], BF16, kind="ExternalInput").ap()
    xk = nc.dram_tensor("xk", [D, T], BF16, kind="ExternalInput").ap()
    xv = nc.dram_tensor("xv", [D, T], BF16, kind="ExternalInput").ap()
    # weights host-preswizzled to [128, chunk, cols] DMA-contiguous layout
    wq = nc.dram_tensor("wq", [128, 8 * W], BF16, kind="ExternalInput").ap()
    wk = nc.dram_tensor("wk", [128, 8 * W], BF16, kind="ExternalInput").ap()
    wv = nc.dram_tensor("wv", [128, 8 * W], BF16, kind="ExternalInput").ap()
    wo = nc.dram_tensor("wo", [128, 2 * D], BF16, kind="ExternalInput").ap()
    bq = nc.dram_tensor("bq", [1, W], BF16, kind="ExternalInput").ap()
    bk = nc.dram_tensor("bk", [1, W], BF16, kind="ExternalInput").ap()
    bv = nc.dram_tensor("bv", [1, W], BF16, kind="ExternalInput").ap()
    ident = nc.dram_tensor("ident", [128, 128], BF16, kind="ExternalInput").ap()
    out = nc.dram_tensor("out", [D, T], F32, kind="ExternalOutput").ap()

    Exp = mybir.ActivationFunctionType.Exp

    with tile.TileContext(nc) as tc:
        with (
            tc.tile_pool(name="persist", bufs=1) as persist,
            tc.tile_pool(name="xpool", bufs=8) as xpool,
            tc.tile_pool(name="upool", bufs=8) as upool,
            tc.tile_pool(name="rpool", bufs=2) as rpool,
            tc.tile_pool(name="opool", bufs=4) as opool,
        ):
            # ---- constants ----
            ones_row = persist.tile([1, 512], BF16, tag="ones_row")
            nc.vector.memset(ones_row, 1.0)
            ident_sb = persist.tile([128, 128], BF16, tag="ident")
            nc.sync.dma_start(out=ident_sb, in_=ident)
            # K=1 broadcast matmul stationary: ones row at partition 64
            bcast1 = persist.tile([65, 64], BF16, tag="bcast1")
            nc.vector.memset(bcast1[64:65, :], 1.0)

            # ---- weights / biases ----
            # K's weights first -- the first matmuls wait only on these
            wk_sb = persist.tile([128, 8, W], BF16, tag="wk")
            nc.sync.dma_start(out=wk_sb, in_=wk.rearrange("p (c j) -> p c j", j=W))
            bk_sb = persist.tile([1, W], BF16, tag="bk")
            nc.sync.dma_start(out=bk_sb, in_=bk)
            wq_sb = persist.tile([128, 8, W], BF16, tag="wq")
            nc.sync.dma_start(out=wq_sb, in_=wq.rearrange("p (c j) -> p c j", j=W))
            bq_sb = persist.tile([1, W], BF16, tag="bq")
            nc.sync.dma_start(out=bq_sb, in_=bq)
            wv_sb = persist.tile([128, 8, W], BF16, tag="wv")
            nc.sync.dma_start(out=wv_sb, in_=wv.rearrange("p (c j) -> p c j", j=W))
            bv_sb = persist.tile([1, W], BF16, tag="bv")
            nc.sync.dma_start(out=bv_sb, in_=bv)
            wo_sb = persist.tile([128, 2, D], BF16, tag="wo")
            nc.sync.dma_start(out=wo_sb, in_=wo.rearrange("p (c e) -> p c e", e=D))

            # ---- persistent activations ----
            qt_sb = persist.tile([128, 2, T], BF16, tag="qt")   # QT [j, t]
            kt_sb = persist.tile([128, 2, T], BF16, tag="kt")   # KT [j, t]
            # V augmented with ones column per head: [k, kt, h, 0:64]=V, [..64]=1
            vaug_sb = persist.tile([128, 16, HPC, Hd + 1], BF16, tag="vaug")
            nc.vector.memset(vaug_sb[:, :, :, 64:65], 1.0)
            otn_sb = persist.tile([128, 2, T], BF16, tag="otn")  # normalized O.T
            vt_sb = persist.tile([128, 2, T], BF16, tag="vt")    # V.T [j, t]
            # raw [O.T ; denom] per block b2 = (pr*4+qt)*2 + m
            oraw_sb = persist.tile([65, 16, 512], F32, tag="oraw")

            # ================= Phase A: projections =================
            with tc.tile_pool(name="psA", bufs=8, space="PSUM") as psA:
                def qk_proj(x_dram, w_sb, b_sb, dst):
                    ps = [psA.tile([128, 512], F32, tag="proj", name=f"proj{i}")
                          for i in range(8)]
                    for c in range(8):
                        xc = xpool.tile([128, T], BF16, tag="x", name="xc")
                        nc.gpsimd.dma_start(out=xc,
                                            in_=x_dram[c * 128:(c + 1) * 128, :])
                        for jt in range(2):
                            for tt in range(4):
                                nc.tensor.matmul(
                                    ps[jt * 4 + tt],
                                    lhsT=w_sb[:, c, jt * 128:(jt + 1) * 128],
                                    rhs=xc[:, tt * 512:(tt + 1) * 512],
                                    start=(c == 0), stop=False,
                                )
                    for jt in range(2):
                        for tt in range(4):
                            p = ps[jt * 4 + tt]
                            nc.tensor.matmul(
                                p, lhsT=b_sb[:, jt * 128:(jt + 1) * 128],
                                rhs=ones_row, start=False, stop=True,
                            )
                            nc.vector.tensor_copy(
                                dst[:, jt, tt * 512:(tt + 1) * 512], p)

                def v_proj():
                    # V.T in [j, t] layout like Q/K (stationary weights, one
                    # pass over xv), then PE transposes into vaug's [t, j].
                    qk_proj(xv, wv_sb, bv_sb, vt_sb)
                    for jt in range(2):
                        for tt in range(16):
                            tp = psA.tile([128, 128], BF16, tag="proj",
                                          name="tp")
                            nc.tensor.transpose(
                                tp, vt_sb[:, jt, tt * 128:(tt + 1) * 128],
                                ident_sb)
                            nc.vector.tensor_copy(
                                vaug_sb[:, tt, 2 * jt:2 * jt + 2, 0:64],
                                tp.rearrange("t (h d) -> t h d", h=2))

                # K then Q so the attention score matmuls + exp can begin
                # while the V projection still runs (V only gates the PV
                # matmuls, which trail by a k tile anyway).
                qk_proj(xk, wk_sb, bk_sb, kt_sb)
                qk_proj(xq, wq_sb, bq_sb, qt_sb)
                v_proj()

            # ====== Phase B/D + fused normalization/output projection ======
            with tc.tile_pool(name="psB", bufs=1, space="PSUM") as psB:
                recips = {}

                def attn_block(pr, qt, fillers=()):
                    fillers = dict(fillers)
                    qsl = slice(qt * 512, (qt + 1) * 512)
                    o_psA = psB.tile([65, 512], F32, tag="oA", bufs=1,
                                     name="o_psA")
                    o_psB = psB.tile([65, 512], F32, tag="oB", bufs=1,
                                     name="o_psB")
                    us = []
                    for kt in range(17):
                        if kt < 16:
                            s_big = psB.tile([128, 2, 512], F32, tag="s",
                                             bufs=3, name="s_big")
                            for m in range(2):
                                po = 64 * m
                                nc.tensor.matmul(
                                    s_big[:, m, :],
                                    lhsT=kt_sb[po:po + 64, pr,
                                               kt * 128:(kt + 1) * 128],
                                    rhs=qt_sb[po:po + 64, pr, qsl],
                                    start=True, stop=True,
                                )
                            u_big = upool.tile([128, 2, 512], BF16, tag="u",
                                               name="u_big")
                            nc.scalar.activation(u_big, s_big, Exp, scale=SCALE)
                            us.append(u_big)
                        if kt >= 1:
                            for m, o_ps in ((0, o_psA), (1, o_psB)):
                                h = 2 * pr + m
                                nc.tensor.matmul(
                                    o_ps,
                                    lhsT=vaug_sb[:, kt - 1, h, :],
                                    rhs=us[kt - 1][:, m, :],
                                    start=(kt == 1), stop=(kt == 16),
                                )
                        # weave prior-tile normalization / projection work
                        # into the loop so ScalarE never starves at block
                        # boundaries
                        if kt in fillers:
                            fillers.pop(kt)()
                    for fn in fillers.values():
                        fn()
                    # fast reciprocal of each denominator row straight from
                    # PSUM (unblocks the woven rb fillers early), bf16 cast
                    # for the broadcast matmul, then stage raw results.
                    # stage raw results; per-head reciprocal of the
                    # denominator row (partition 64), bf16 cast for the
                    # broadcast matmul. Runs on DVE slack during the next
                    # block; the woven rb fillers are scheduled late enough.
                    for m, o_ps in ((0, o_psA), (1, o_psB)):
                        b2 = (pr * 4 + qt) * 2 + m
                        nc.vector.tensor_copy(oraw_sb[:, b2, :], o_ps)
                        rtb = rpool.tile([65, 512], BF16, tag="rtb", bufs=8,
                                         name="rtb")
                        with nc.allow_low_precision(
                                reason="1/denom bf16; ample for softmax"):
                            nc.vector.reciprocal(rtb[64:65, :],
                                                 oraw_sb[64:65, b2, :])
                        recips[b2] = rtb

                def norm_pieces(qt):
                    # normalize O.T for q tile qt: 4 filler closures
                    qsl = slice(qt * 512, (qt + 1) * 512)

                    def piece(pr, m):
                        def run():
                            b2 = (pr * 4 + qt) * 2 + m
                            rb_ps = psB.tile([64, 512], F32, tag="s", bufs=3,
                                             name="rb_ps")
                            nc.tensor.matmul(
                                rb_ps, lhsT=bcast1[64:65, :],
                                rhs=recips[b2][64:65, :],
                                start=True, stop=True)
                            rb_sb = rpool.tile([64, 512], F32, tag="rbs",
                                               name="rb_sb")
                            nc.vector.tensor_copy(rb_sb, rb_ps)
                            if m == 0:
                                nc.vector.tensor_mul(
                                    otn_sb[0:64, pr, qsl],
                                    oraw_sb[0:64, b2, :], rb_sb)
                            else:
                                otnB = rpool.tile([64, 512], BF16, tag="otnB",
                                                  name="otnB")
                                nc.vector.tensor_mul(
                                    otnB, oraw_sb[0:64, b2, :], rb_sb)
                                nc.sync.dma_start(
                                    out=otn_sb[64:128, pr, qsl], in_=otnB)
                        return run
                    # later slots: (pr=1) reciprocals are issued at the
                    # immediately preceding block boundary and need ~7us
                    return [(9, piece(0, 0)), (11, piece(0, 1)),
                            (13, piece(1, 0)), (15, piece(1, 1))]

                def proj_pieces(qt):
                    # output projection for q tile qt: 8 filler closures
                    qsl = slice(qt * 512, (qt + 1) * 512)

                    def piece(et):
                        def run():
                            e_ps = psB.tile([128, 512], F32, tag="s", bufs=3,
                                            name="e_ps")
                            for jc in range(2):
                                nc.tensor.matmul(
                                    e_ps,
                                    lhsT=wo_sb[:, jc, et * 128:(et + 1) * 128],
                                    rhs=otn_sb[:, jc, qsl],
                                    start=(jc == 0), stop=(jc == 1),
                                )
                            stg = opool.tile([128, 512], F32, tag="ostg",
                                             name="stg")
                            nc.vector.tensor_copy(stg, e_ps)
                            nc.sync.dma_start(
                                out=out[et * 128:(et + 1) * 128, qsl], in_=stg)
                        return run
                    return [(2 * et + 2, piece(et)) for et in range(8)]

                for qt in range(4):
                    attn_block(0, qt,
                               fillers=norm_pieces(qt - 1) if qt >= 1 else ())
                    attn_block(1, qt,
                               fillers=proj_pieces(qt - 1) if qt >= 1 else ())
                for _, f in norm_pieces(3):
                    f()
                for _, f in proj_pieces(3):
                    f()

    nc.finalize()
    return nc


_NC_CACHE = None


def _get_nc():
    global _NC_CACHE
    if _NC_CACHE is None:
        _NC_CACHE = build_nc()
    return _NC_CACHE


def _swz(wT):
    """[C*128, cols] -> DMA-contiguous [128, C*cols] (partition-major)."""
    C = wT.shape[0] // 128
    return np.ascontiguousarray(
        wT.reshape(C, 128, -1).swapaxes(0, 1).reshape(128, -1)).astype(bf16)


def make_in_maps(query, key, value, wq, bq, wk, bk, wv, bv, wo, bo):
    in_maps = []
    for c in range(N_CORES):
        b, hg = divmod(c, HPC)
        sl = slice(hg * W, (hg + 1) * W)
        in_maps.append({
            "xq": np.ascontiguousarray(np.asarray(query[b]).T).astype(bf16),
            "xk": np.ascontiguousarray(np.asarray(key[b]).T).astype(bf16),
            "xv": np.ascontiguousarray(np.asarray(value[b]).T).astype(bf16),
            "wq": _swz(np.asarray(wq)[sl].T),
            "wk": _swz(np.asarray(wk)[sl].T),
            "wv": _swz(np.asarray(wv)[sl].T),
            "wo": _swz(np.asarray(wo)[:, sl].T),
            "bq": np.asarray(bq)[sl].reshape(1, W).astype(bf16),
            "bk": np.asarray(bk)[sl].reshape(1, W).astype(bf16),
            "bv": np.asarray(bv)[sl].reshape(1, W).astype(bf16),
            "ident": np.eye(128, dtype=np.float32).astype(bf16),
        })
    return in_maps


def combine_outputs(outs, bo):
    full = np.zeros((B, T, D), np.float32)
    for c in range(N_CORES):
        b = c // HPC
        full[b] += outs[c].T
    full += np.asarray(bo, np.float32)[None, None, :]
    return full


def kernel(query, key, value, wq, bq, wk, bk, wv, bv, wo, bo):
    nc = _get_nc()
    in_maps = make_in_maps(query, key, value, wq, bq, wk, bk, wv, bv, wo, bo)
    res = run_bass_kernel_spmd(nc, in_maps, list(range(N_CORES)))
    outs = [np.asarray(res.results[c]["out"]) for c in range(N_CORES)]
    return combine_outputs(outs, bo)



# revision 4
# speedup vs baseline: 1.0011x; 1.0011x over previous
"""Multi-head attention (B=2, T=2048, D=1024, H=16) on 8 TRN2 NeuronCores.

Sharding: 2D (batch x head-group). Core c handles batch b = c // 4 and head
group hg = c % 4 (4 heads = 256 channels of the projected dim).

Single software-pipelined phase per core (no projection/attention barrier):
  - Inputs stream in per 512-column t-tile ([128, 8, 512] staged, 1MB DMAs),
    highest-priority first (wk, xk-t0, wq, xq-t0, ...), so the first score
    matmuls run ~9us in and ScalarE (the exp bottleneck, ~128us of ACTIVATE)
    stays saturated to the end.
  - A static step scheduler walks 8 blocks x 16 key-tiles. Each step: two
    row-concurrent score matmuls (heads at partition 0/64), one 1024-elem
    exp ACTIVATE, then PV chunks and filler units (remaining projections,
    normalization, output projection) drained from queues sized to keep
    TensorE just under the ACT pace.
  - PV trails exp by up to ~2 blocks during the ramp (u tiles ring-buffered
    in SBUF); the softmax denominator rides the PV matmul as row 64 of the
    [V|1]-augmented weights.
  - V is projected in [j, t] layout like Q/K, then moved to [t, j] via
    HWDGE dma_start_transpose (off the PE) and copied into the augmented
    V tiles by DVE.
  - Reciprocals use reciprocal_approx_fast (single DVE op) instead of the
    iterative reciprocal; 1/denom is broadcast over 64 partitions by a K=1
    ones matmul, normalization multiplies read O straight from PSUM.

PSUM (8 banks): scores [128,2,512] x2 (4) + O accumulators [65,512] x2 (2)
+ aux ring [128,512] x2 (2, shared by projection / rb / out-proj tiles).

All shapes hardcoded. kernel() takes full inputs, returns [2, 2048, 1024].
"""

import numpy as np
import ml_dtypes

import concourse.bass as bass
import concourse.bacc as bacc
import concourse.mybir as mybir
import concourse.tile as tile
from concourse.bass_utils import run_bass_kernel_spmd

B, T, D, H, Hd = 2, 2048, 1024, 16, 64
HPC = 4          # heads per core
W = HPC * Hd     # 256 projected channels per core
SCALE = Hd ** -0.5
N_CORES = 8
NT = 4           # 512-wide t-tiles
NC = 8           # 128-deep contraction chunks

BF16 = mybir.dt.bfloat16
F32 = mybir.dt.float32
bf16 = ml_dtypes.bfloat16

BLOCKS = [(0, 0), (1, 0), (0, 1), (1, 1), (0, 2), (1, 2), (0, 3), (1, 3)]


def build_nc():
    nc = bacc.Bacc("TRN2", target_bir_lowering=False, debug=False)

    xq = nc.dram_tensor("xq", [128, NT * NC * 512], BF16, kind="ExternalInput").ap()
    xk = nc.dram_tensor("xk", [128, NT * NC * 512], BF16, kind="ExternalInput").ap()
    xv = nc.dram_tensor("xv", [128, NT * NC * 512], BF16, kind="ExternalInput").ap()
    wq = nc.dram_tensor("wq", [128, NC * W], BF16, kind="ExternalInput").ap()
    wk = nc.dram_tensor("wk", [128, NC * W], BF16, kind="ExternalInput").ap()
    wv = nc.dram_tensor("wv", [128, NC * W], BF16, kind="ExternalInput").ap()
    wo = nc.dram_tensor("wo", [128, 2 * D], BF16, kind="ExternalInput").ap()
    bq = nc.dram_tensor("bq", [128, 2], F32, kind="ExternalInput").ap()
    bk = nc.dram_tensor("bk", [128, 2], F32, kind="ExternalInput").ap()
    bv = nc.dram_tensor("bv", [128, 2], F32, kind="ExternalInput").ap()
    out = nc.dram_tensor("out", [D, T], F32, kind="ExternalOutput").ap()

    xq_t = xq.rearrange("p (t c q) -> p t c q", c=NC, q=512)
    xk_t = xk.rearrange("p (t c q) -> p t c q", c=NC, q=512)
    xv_t = xv.rearrange("p (t c q) -> p t c q", c=NC, q=512)

    Exp = mybir.ActivationFunctionType.Exp

    with tile.TileContext(nc) as tc:
        with (
            tc.tile_pool(name="persist", bufs=1) as persist,
            tc.tile_pool(name="xst", bufs=4) as xst,
            tc.tile_pool(name="upool", bufs=48) as upool,
            tc.tile_pool(name="small", bufs=2) as small,
            tc.tile_pool(name="opool", bufs=4) as stgp,
            tc.tile_pool(name="ps", bufs=1, space="PSUM") as ps,
        ):
            # ---- persistent constants / weights / activations ----
            bcast1 = persist.tile([65, 64], BF16, tag="bcast1")
            nc.vector.memset(bcast1[64:65, :], 1.0)

            wk_sb = persist.tile([128, NC, W], BF16, tag="wk")
            wq_sb = persist.tile([128, NC, W], BF16, tag="wq")
            wv_sb = persist.tile([128, NC, W], BF16, tag="wv")
            wo_sb = persist.tile([128, 2, D], BF16, tag="wo")
            bq_sb = persist.tile([128, 2], F32, tag="bq")
            bk_sb = persist.tile([128, 2], F32, tag="bk")
            bv_sb = persist.tile([128, 2], F32, tag="bv")

            qt_sb = persist.tile([128, 2, T], BF16, tag="qt")   # Q.T [j, t]
            kt_sb = persist.tile([128, 2, T], BF16, tag="kt")   # K.T [j, t]
            otn_sb = persist.tile([128, 2, T], BF16, tag="otn")  # normalized O.T
            # V augmented with ones column per head: [k, kt16, h4, 0:64]=V
            vaug = persist.tile([128, 16, HPC, Hd + 1], BF16, tag="vaug")
            nc.vector.memset(vaug[:, :, :, 64:65], 1.0)

            # ---- DMA issue, priority order (sync ring is FIFO) ----
            x_tiles = {}

            def dma_w(dst, src):
                nc.sync.dma_start(out=dst, in_=src)

            def dma_x(name, dram, tt):
                t = xst.tile([128, NC, 512], BF16, tag="xst", name="xst")
                nc.sync.dma_start(out=t, in_=dram[:, tt])
                x_tiles[(name, tt)] = t

            dma_w(wk_sb, wk.rearrange("p (c j) -> p c j", j=W))
            dma_w(bk_sb, bk)
            dma_x("k", xk_t, 0)
            dma_w(wq_sb, wq.rearrange("p (c j) -> p c j", j=W))
            dma_w(bq_sb, bq)
            dma_x("q", xq_t, 0)
            dma_x("k", xk_t, 1)
            dma_x("k", xk_t, 2)
            dma_x("k", xk_t, 3)
            dma_x("q", xq_t, 1)
            dma_w(bv_sb, bv)
            dma_w(wv_sb, wv.rearrange("p (c j) -> p c j", j=W))
            dma_x("v", xv_t, 0)
            dma_x("v", xv_t, 1)
            dma_x("v", xv_t, 2)
            dma_x("v", xv_t, 3)
            dma_w(wo_sb, wo.rearrange("p (c e) -> p c e", e=D))
            dma_x("q", xq_t, 2)
            dma_x("q", xq_t, 3)

            # ---- projection units ----
            def qk_proj(name, w_sb, b_sb, dst, tt):
                # one 512-wide t-tile, both j-halves, serial on the aux ring
                xc = x_tiles[(name, tt)]
                for jt in range(2):
                    p = ps.tile([128, 512], F32, tag="aux", bufs=2, name="aux")
                    for c in range(NC):
                        nc.tensor.matmul(
                            p, lhsT=w_sb[:, c, jt * 128:(jt + 1) * 128],
                            rhs=xc[:, c, :], start=(c == 0), stop=(c == NC - 1))
                    nc.vector.tensor_scalar_add(
                        dst[:, jt, tt * 512:(tt + 1) * 512], p,
                        b_sb[:, jt:jt + 1])

            def v_proj(jt, tt):
                # VT tile -> DMA-transpose -> vaug slices
                xc = x_tiles[("v", tt)]
                p = ps.tile([128, 512], F32, tag="aux", bufs=2, name="aux")
                for c in range(NC):
                    nc.tensor.matmul(
                        p, lhsT=wv_sb[:, c, jt * 128:(jt + 1) * 128],
                        rhs=xc[:, c, :], start=(c == 0), stop=(c == NC - 1))
                vt = small.tile([128, 512], BF16, tag="vt", bufs=3, name="vt")
                nc.vector.tensor_scalar_add(vt, p, bv_sb[:, jt:jt + 1])
                for i in range(4):
                    tp = small.tile([128, 128], BF16, tag="vtp", bufs=3,
                                    name="vtp")
                    nc.sync.dma_start_transpose(
                        out=tp, in_=vt[:, i * 128:(i + 1) * 128])
                    nc.vector.tensor_copy(
                        vaug[:, tt * 4 + i, 2 * jt:2 * jt + 2, 0:64],
                        tp.rearrange("t (h d) -> t h d", h=2))

            # ---- attention pieces ----
            us = {}          # (block, kt) -> u tile
            cur_o = {}       # block -> (oA, oB)
            otn_ready = {}   # block index -> True once norm emitted

            def scores_exp(b, kt):
                pr, qt = BLOCKS[b]
                qsl = slice(qt * 512, (qt + 1) * 512)
                s = ps.tile([128, 2, 512], F32, tag="s", bufs=2, name="s")
                for m in range(2):
                    po = 64 * m
                    nc.tensor.matmul(
                        s[:, m, :],
                        lhsT=kt_sb[po:po + 64, pr, kt * 128:(kt + 1) * 128],
                        rhs=qt_sb[po:po + 64, pr, qsl],
                        start=True, stop=True)
                u = upool.tile([128, 2, 512], BF16, tag="u", name="u")
                nc.scalar.activation(u, s, Exp, scale=SCALE)
                us[(b, kt)] = u

            def pv_chunk(b, kt):
                pr, qt = BLOCKS[b]
                if kt == 0:
                    cur_o[b] = (
                        ps.tile([65, 512], F32, tag="oA", bufs=1, name="oA"),
                        ps.tile([65, 512], F32, tag="oB", bufs=1, name="oB"))
                oa, ob = cur_o[b]
                u = us.pop((b, kt))
                for m, o_ps in ((0, oa), (1, ob)):
                    nc.tensor.matmul(
                        o_ps, lhsT=vaug[:, kt, 2 * pr + m, :],
                        rhs=u[:, m, :],
                        start=(kt == 0), stop=(kt == 15))

            def norm(b):
                pr, qt = BLOCKS[b]
                qsl = slice(qt * 512, (qt + 1) * 512)
                for m, o_ps in ((0, cur_o[b][0]), (1, cur_o[b][1])):
                    # approx_fast breaks on single-row slices; run full-tile
                    # (same per-lane cost), only row 64 (denom) is used.
                    rf = small.tile([65, 512], F32, tag="rf", name="rf")
                    nc.vector.reciprocal_approx_fast(out=rf, in_=o_ps)
                    r16 = small.tile([65, 512], BF16, tag="r16", name="r16")
                    nc.vector.tensor_copy(r16[64:65, :], rf[64:65, :])
                    rbp = ps.tile([64, 512], F32, tag="aux", bufs=2, name="aux")
                    nc.tensor.matmul(rbp, lhsT=bcast1[64:65, :],
                                     rhs=r16[64:65, :], start=True, stop=True)
                    rbs = small.tile([64, 512], F32, tag="rbs", name="rbs")
                    nc.vector.tensor_copy(rbs, rbp)
                    if m == 0:
                        nc.vector.tensor_mul(
                            otn_sb[0:64, pr, qsl], o_ps[0:64, :], rbs)
                    else:
                        otnB = small.tile([64, 512], BF16, tag="otnB",
                                          name="otnB")
                        nc.vector.tensor_mul(otnB, o_ps[0:64, :], rbs)
                        nc.sync.dma_start(out=otn_sb[64:128, pr, qsl],
                                          in_=otnB)
                otn_ready[b] = True

            def out_proj(qt, et):
                qsl = slice(qt * 512, (qt + 1) * 512)
                e = ps.tile([128, 512], F32, tag="aux", bufs=2, name="aux")
                for jc in range(2):
                    nc.tensor.matmul(
                        e, lhsT=wo_sb[:, jc, et * 128:(et + 1) * 128],
                        rhs=otn_sb[:, jc, qsl],
                        start=(jc == 0), stop=(jc == 1))
                stg = stgp.tile([128, 512], F32, tag="stg", name="stg")
                nc.vector.tensor_copy(stg, e)
                nc.gpsimd.dma_start(out=out[et * 128:(et + 1) * 128, qsl],
                                    in_=stg)

            # ---- prologue projections ----
            qk_proj("k", wk_sb, bk_sb, kt_sb, 0)
            qk_proj("q", wq_sb, bq_sb, qt_sb, 0)

            # ---- static filler schedule: global step -> closures ----
            fillers = {
                2: [lambda: qk_proj("k", wk_sb, bk_sb, kt_sb, 1)],
                5: [lambda: qk_proj("k", wk_sb, bk_sb, kt_sb, 2)],
                8: [lambda: qk_proj("k", wk_sb, bk_sb, kt_sb, 3)],
                18: [lambda: qk_proj("q", wq_sb, bq_sb, qt_sb, 1)],
                22: [lambda: v_proj(0, 0)],
                24: [lambda: v_proj(0, 1)],
                26: [lambda: v_proj(0, 2)],
                28: [lambda: v_proj(0, 3)],
                30: [lambda: v_proj(1, 0)],
                32: [lambda: v_proj(1, 1)],
                34: [lambda: v_proj(1, 2)],
                37: [lambda: v_proj(1, 3)],
                40: [lambda: qk_proj("q", wq_sb, bq_sb, qt_sb, 2)],
                56: [lambda: qk_proj("q", wq_sb, bq_sb, qt_sb, 3)],
            }

            # PV chunk queue with readiness: vaug jt0 usable ~g36, jt1 ~g40
            pv_queue = [(b, kt) for b in range(8) for kt in range(16)]
            pv_pos = 0

            def pv_ready_g(b):
                pr, _ = BLOCKS[b]
                return 36 if pr == 0 else 40

            op_queue = []       # pending out_proj units
            qt_done = set()

            def drain(g, budget):
                nonlocal pv_pos
                n = 0
                while pv_pos < len(pv_queue) and n < budget:
                    pb, pkt = pv_queue[pv_pos]
                    if g <= 16 * pb + pkt:      # u not produced yet
                        break
                    if g < pv_ready_g(pb):      # vaug not ready
                        break
                    pv_pos += 1
                    pv_chunk(pb, pkt)
                    n += 1
                    if pkt == 15:
                        norm(pb)
                        pr, qt = BLOCKS[pb]
                        if pb % 2 == 1 and qt not in qt_done:
                            qt_done.add(qt)
                            op_queue.extend(
                                (qt, et) for et in range(NC))
                return n

            # ---- main loop ----
            for g in range(128):
                b, s = g // 16, g % 16
                scores_exp(b, s)
                fl = fillers.get(g, [])
                for f in fl:
                    f()
                backlog = (g - 16) - pv_pos  # rough chunks-behind measure
                if fl:
                    drain(g, 1)
                elif op_queue:
                    drain(g, 1 if backlog < 16 else 2)
                    qt, et = op_queue.pop(0)
                    out_proj(qt, et)
                else:
                    drain(g, 3 if backlog > 16 else 2)

            # ---- tail drain ----
            g = 128
            while pv_pos < len(pv_queue):
                drained = drain(g, 4)
                g += max(drained, 1)
            for qt, et in op_queue:
                out_proj(qt, et)

    nc.finalize()
    return nc


_NC_CACHE = None


def _get_nc():
    global _NC_CACHE
    if _NC_CACHE is None:
        _NC_CACHE = build_nc()
    return _NC_CACHE


def _swz(wT):
    """[C*128, cols] -> DMA-contiguous [128, C*cols] (partition-major)."""
    C = wT.shape[0] // 128
    return np.ascontiguousarray(
        wT.reshape(C, 128, -1).swapaxes(0, 1).reshape(128, -1)).astype(bf16)


def _xprep(x):
    """[T, D] -> [128, NT*NC*512]: tile tt gives [128(p), NC(c), 512(t)]
    where element (p, c, t) = x[tt*512 + t, c*128 + p]."""
    xT = np.asarray(x).T                      # [D, T]
    a = xT.reshape(NC, 128, NT, 512)          # [c, p, tt, t]
    a = a.transpose(1, 2, 0, 3)               # [p, tt, c, t]
    return np.ascontiguousarray(a.reshape(128, -1)).astype(bf16)


def _bcol(b, sl):
    return np.ascontiguousarray(
        np.asarray(b)[sl].reshape(2, 128).T).astype(np.float32)


def make_in_maps(query, key, value, wq, bq, wk, bk, wv, bv, wo, bo):
    xq_b = [_xprep(query[b]) for b in range(B)]
    xk_b = [_xprep(key[b]) for b in range(B)]
    xv_b = [_xprep(value[b]) for b in range(B)]
    in_maps = []
    for c in range(N_CORES):
        b, hg = divmod(c, HPC)
        sl = slice(hg * W, (hg + 1) * W)
        in_maps.append({
            "xq": xq_b[b],
            "xk": xk_b[b],
            "xv": xv_b[b],
            "wq": _swz(np.asarray(wq)[sl].T),
            "wk": _swz(np.asarray(wk)[sl].T),
            "wv": _swz(np.asarray(wv)[sl].T),
            "wo": _swz(np.asarray(wo)[:, sl].T),
            "bq": _bcol(bq, sl),
            "bk": _bcol(bk, sl),
            "bv": _bcol(bv, sl),
        })
    return in_maps


def combine_outputs(outs, bo):
    full = np.zeros((B, T, D), np.float32)
    for c in range(N_CORES):
        b = c // HPC
        full[b] += outs[c].T
    full += np.asarray(bo, np.float32)[None, None, :]
    return full


def kernel(query, key, value, wq, bq, wk, bk, wv, bv, wo, bo):
    nc = _get_nc()
    in_maps = make_in_maps(query, key, value, wq, bq, wk, bk, wv, bv, wo, bo)
    res = run_bass_kernel_spmd(nc, in_maps, list(range(N_CORES)))
    outs = [np.asarray(res.results[c]["out"]) for c in range(N_CORES)]
    return combine_outputs(outs, bo)


# revision 7
# speedup vs baseline: 1.1231x; 1.1219x over previous
"""Multi-head attention (B=2, T=2048, D=1024, H=16) on 8 TRN2 NeuronCores.

Sharding: 2D (batch x head-group). Core c handles batch b = c // 4 and head
group hg = c % 4 (4 heads = 256 channels of the projected dim).

Single software-pipelined phase per core (no projection/attention barrier):
  - Warmup at t=0: dummy matmuls un-throttle the PE HAM clock gate and a
    dummy exp loads the ACT table set before real data arrives.
  - Inputs stream in per 512-column t-tile ([128, 8, 512] staged, 1MB DMAs)
    in priority order; the first x tile rides the SWDGE ring concurrently
    with weights on the HWDGE ring. Q/K projections are split by j-half so
    only the jt=0 halves gate the first score matmuls (~10us in); ScalarE
    (the exp bottleneck, ~128us of ACTIVATE) then stays busy to the end.
  - V is projected directly into [t, j] layout (xv chunks as the stationary
    operand, N=256) - no transposes anywhere - and lands in the
    [V|1]-augmented PV weight tiles via one DVE add (bias broadcast from a
    host-replicated tile).
  - A static step scheduler walks 8 blocks x 16 key-tiles. Each step: two
    row-concurrent score matmuls (heads at partition offsets 0/64), one
    1024-element exp ACTIVATE, then PV chunks and filler units (remaining
    projections, normalization, output projection) drained from queues
    sized to keep TensorE just under the ACT pace. PV trails exp by up to
    ~2 blocks during the ramp (u tiles ring-buffered in SBUF); the softmax
    denominator rides the PV matmul as row 64 of the augmented weights.
  - Reciprocals use reciprocal_approx_fast (single custom-DVE op, full-tile
    because the op mishandles single-row slices); 1/denom is broadcast over
    64 partitions by a K=1 ones matmul; normalization multiplies read O
    straight from PSUM. Output projection tiles DMA out on the HWDGE ring
    (free after the input stream) as soon as they are produced.

PSUM (8 banks): scores [128,2,512] x2 (4) + O accumulators [65,512] x2 (2)
+ aux ring [128,512] x2 (2, shared by projection / rb / out-proj tiles).

All shapes hardcoded. kernel() takes full inputs, returns [2, 2048, 1024].
"""

import numpy as np
import ml_dtypes

import concourse.bass as bass
import concourse.bacc as bacc
import concourse.mybir as mybir
import concourse.tile as tile
from concourse.bass_utils import run_bass_kernel_spmd

B, T, D, H, Hd = 2, 2048, 1024, 16, 64
HPC = 4          # heads per core
W = HPC * Hd     # 256 projected channels per core
SCALE = Hd ** -0.5
N_CORES = 8
NT = 4           # 512-wide t-tiles
NC = 8           # 128-deep contraction chunks

BF16 = mybir.dt.bfloat16
F32 = mybir.dt.float32
bf16 = ml_dtypes.bfloat16

BLOCKS = [(0, 0), (1, 0), (0, 1), (1, 1), (0, 2), (1, 2), (0, 3), (1, 3)]


def build_nc():
    nc = bacc.Bacc("TRN2", target_bir_lowering=False, debug=False)

    xq = nc.dram_tensor("xq", [128, NT * NC * 512], BF16, kind="ExternalInput").ap()
    xk = nc.dram_tensor("xk", [128, NT * NC * 512], BF16, kind="ExternalInput").ap()
    xv = nc.dram_tensor("xv", [128, NT * NC * 512], BF16, kind="ExternalInput").ap()
    wq = nc.dram_tensor("wq", [128, NC * W], BF16, kind="ExternalInput").ap()
    wk = nc.dram_tensor("wk", [128, NC * W], BF16, kind="ExternalInput").ap()
    wv = nc.dram_tensor("wv", [128, NC * W], BF16, kind="ExternalInput").ap()
    wo = nc.dram_tensor("wo", [128, 2 * D], BF16, kind="ExternalInput").ap()
    bq = nc.dram_tensor("bq", [128, 2], F32, kind="ExternalInput").ap()
    bk = nc.dram_tensor("bk", [128, 2], F32, kind="ExternalInput").ap()
    bvb = nc.dram_tensor("bvb", [128, 256], F32, kind="ExternalInput").ap()
    out = nc.dram_tensor("out", [D, T], F32, kind="ExternalOutput").ap()

    xq_t = xq.rearrange("p (t c q) -> p t c q", c=NC, q=512)
    xk_t = xk.rearrange("p (t c q) -> p t c q", c=NC, q=512)
    xv_t = xv.rearrange("p (t c q) -> p t c q", c=NC, q=512)

    Exp = mybir.ActivationFunctionType.Exp
    Add = mybir.AluOpType.add

    with tile.TileContext(nc) as tc:
        with (
            tc.tile_pool(name="persist", bufs=1) as persist,
            tc.tile_pool(name="xst", bufs=4) as xst,
            tc.tile_pool(name="upool", bufs=48) as upool,
            tc.tile_pool(name="small", bufs=2) as small,
            tc.tile_pool(name="stgp", bufs=4) as stgp,
            tc.tile_pool(name="ps", bufs=1, space="PSUM") as ps,
        ):
            # ---- warmup: HAM un-throttle + ACT table load ----
            bcast1 = persist.tile([65, 64], BF16, tag="bcast1")
            nc.vector.memset(bcast1, 1.0)
            wdum = persist.tile([64, 512], BF16, tag="wdum")
            nc.vector.memset(wdum, 1.0)
            nc.scalar.activation(wdum[:, 0:64], bcast1[0:64, :], Exp,
                                 scale=0.1)
            wps = ps.tile([64, 512], F32, tag="aux", bufs=2, name="aux")
            for i in range(10):
                nc.tensor.matmul(wps, lhsT=bcast1[0:64, :], rhs=wdum,
                                 start=True, stop=True)

            # ---- persistent weights / activations ----
            wk_sb = persist.tile([128, NC, W], BF16, tag="wk")
            wq_sb = persist.tile([128, NC, W], BF16, tag="wq")
            wv_sb = persist.tile([128, NC, W], BF16, tag="wv")
            wo_sb = persist.tile([128, 2, D], BF16, tag="wo")
            bq_sb = persist.tile([128, 2], F32, tag="bq")
            bk_sb = persist.tile([128, 2], F32, tag="bk")
            bvb_sb = persist.tile([128, 256], F32, tag="bvb")

            qt_sb = persist.tile([128, 2, T], BF16, tag="qt")   # Q.T [j, t]
            kt_sb = persist.tile([128, 2, T], BF16, tag="kt")   # K.T [j, t]
            otn_sb = persist.tile([128, 2, T], BF16, tag="otn")  # normalized O.T
            # V augmented with ones column per head: [k, kt16, h4, 0:64]=V
            vaug = persist.tile([128, 16, HPC, Hd + 1], BF16, tag="vaug")
            nc.vector.memset(vaug[:, :, :, 64:65], 1.0)

            # ---- DMA issue, priority order ----
            # first x tile on the SWDGE (gpsimd) ring, concurrent with
            # weights on the HWDGE (sync) ring; rest in priority order.
            x_tiles = {}

            def dma_x(name, dram, tt, eng):
                t = xst.tile([128, NC, 512], BF16, tag="xst", name="xst")
                eng.dma_start(out=t, in_=dram[:, tt])
                x_tiles[(name, tt)] = t

            dma_x("k", xk_t, 0, nc.gpsimd)
            nc.sync.dma_start(out=wk_sb, in_=wk.rearrange("p (c j) -> p c j", j=W))
            nc.sync.dma_start(out=bk_sb, in_=bk)
            nc.sync.dma_start(out=bq_sb, in_=bq)
            dma_x("q", xq_t, 0, nc.gpsimd)
            nc.sync.dma_start(out=wq_sb, in_=wq.rearrange("p (c j) -> p c j", j=W))
            dma_x("k", xk_t, 1, nc.sync)
            nc.gpsimd.dma_start(out=wv_sb, in_=wv.rearrange("p (c j) -> p c j", j=W))
            nc.gpsimd.dma_start(out=bvb_sb, in_=bvb)
            dma_x("k", xk_t, 2, nc.sync)
            dma_x("k", xk_t, 3, nc.gpsimd)
            dma_x("v", xv_t, 0, nc.sync)
            dma_x("v", xv_t, 1, nc.gpsimd)
            dma_x("v", xv_t, 2, nc.sync)
            dma_x("v", xv_t, 3, nc.gpsimd)
            dma_x("q", xq_t, 1, nc.sync)
            nc.gpsimd.dma_start(out=wo_sb, in_=wo.rearrange("p (c e) -> p c e", e=D))
            dma_x("q", xq_t, 2, nc.sync)
            dma_x("q", xq_t, 3, nc.gpsimd)

            # ---- projection units ----
            def qk_proj(name, w_sb, b_sb, dst, jt, tt):
                # one 512-wide t-tile, one j-half
                xc = x_tiles[(name, tt)]
                p = ps.tile([128, 512], F32, tag="aux", bufs=2, name="aux")
                for c in range(NC):
                    nc.tensor.matmul(
                        p, lhsT=w_sb[:, c, jt * 128:(jt + 1) * 128],
                        rhs=xc[:, c, :], start=(c == 0), stop=(c == NC - 1))
                nc.vector.tensor_scalar_add(
                    dst[:, jt, tt * 512:(tt + 1) * 512], p, b_sb[:, jt:jt + 1])

            def v_proj(kt):
                # V in [t, j] layout directly: xv chunks stationary, N=256
                xc = x_tiles[("v", kt // 4)]
                tl = kt % 4
                p = ps.tile([128, 256], F32, tag="aux", bufs=2, name="aux")
                for c in range(NC):
                    nc.tensor.matmul(
                        p, lhsT=xc[:, c, tl * 128:(tl + 1) * 128],
                        rhs=wv_sb[:, c, :], start=(c == 0), stop=(c == NC - 1))
                nc.vector.tensor_tensor(
                    out=vaug[:, kt, :, 0:64],
                    in0=p.rearrange("p (h d) -> p h d", h=HPC),
                    in1=bvb_sb.rearrange("p (h d) -> p h d", h=HPC), op=Add)

            # ---- attention pieces ----
            us = {}          # (block, kt) -> u tile
            cur_o = {}       # block -> (oA, oB)

            def scores_exp(b, kt):
                pr, qt = BLOCKS[b]
                qsl = slice(qt * 512, (qt + 1) * 512)
                s = ps.tile([128, 2, 512], F32, tag="s", bufs=2, name="s")
                for m in range(2):
                    po = 64 * m
                    nc.tensor.matmul(
                        s[:, m, :],
                        lhsT=kt_sb[po:po + 64, pr, kt * 128:(kt + 1) * 128],
                        rhs=qt_sb[po:po + 64, pr, qsl],
                        start=True, stop=True)
                u = upool.tile([128, 2, 512], BF16, tag="u", name="u")
                nc.scalar.activation(u, s, Exp, scale=SCALE)
                us[(b, kt)] = u

            def pv_chunk(b, kt):
                pr, qt = BLOCKS[b]
                if kt == 0:
                    cur_o[b] = (
                        ps.tile([65, 512], F32, tag="oA", bufs=1, name="oA"),
                        ps.tile([65, 512], F32, tag="oB", bufs=1, name="oB"))
                oa, ob = cur_o[b]
                u = us.pop((b, kt))
                for m, o_ps in ((0, oa), (1, ob)):
                    nc.tensor.matmul(
                        o_ps, lhsT=vaug[:, kt, 2 * pr + m, :],
                        rhs=u[:, m, :],
                        start=(kt == 0), stop=(kt == 15))

            def norm(b):
                pr, qt = BLOCKS[b]
                qsl = slice(qt * 512, (qt + 1) * 512)
                for m, o_ps in ((0, cur_o[b][0]), (1, cur_o[b][1])):
                    # approx_fast breaks on single-row slices; run full-tile
                    # (same per-lane cost), only row 64 (denom) is used.
                    rf = small.tile([65, 512], F32, tag="rf", name="rf")
                    nc.vector.reciprocal_approx_fast(out=rf, in_=o_ps)
                    r16 = small.tile([65, 512], BF16, tag="r16", name="r16")
                    nc.vector.tensor_copy(r16[64:65, :], rf[64:65, :])
                    rbp = ps.tile([64, 512], F32, tag="aux", bufs=2, name="aux")
                    nc.tensor.matmul(rbp, lhsT=bcast1[64:65, :],
                                     rhs=r16[64:65, :], start=True, stop=True)
                    rbs = small.tile([64, 512], F32, tag="rbs", name="rbs")
                    nc.vector.tensor_copy(rbs, rbp)
                    if m == 0:
                        nc.vector.tensor_mul(
                            otn_sb[0:64, pr, qsl], o_ps[0:64, :], rbs)
                    else:
                        otnB = small.tile([64, 512], BF16, tag="otnB",
                                          name="otnB")
                        nc.vector.tensor_mul(otnB, o_ps[0:64, :], rbs)
                        nc.gpsimd.dma_start(out=otn_sb[64:128, pr, qsl],
                                            in_=otnB)

            def out_proj(qt, et):
                qsl = slice(qt * 512, (qt + 1) * 512)
                e = ps.tile([128, 512], F32, tag="aux", bufs=2, name="aux")
                for jc in range(2):
                    nc.tensor.matmul(
                        e, lhsT=wo_sb[:, jc, et * 128:(et + 1) * 128],
                        rhs=otn_sb[:, jc, qsl],
                        start=(jc == 0), stop=(jc == 1))
                stg = stgp.tile([128, 512], F32, tag="stg", name="stg")
                nc.vector.tensor_copy(stg, e)
                nc.sync.dma_start(out=out[et * 128:(et + 1) * 128, qsl],
                                  in_=stg)

            # ---- prologue projections (jt=0 only; jt=1 are fillers) ----
            qk_proj("k", wk_sb, bk_sb, kt_sb, 0, 0)
            qk_proj("q", wq_sb, bq_sb, qt_sb, 0, 0)

            # ---- static filler schedule: global step -> closures ----
            fillers = {
                1: [lambda: qk_proj("k", wk_sb, bk_sb, kt_sb, 0, 1)],
                2: [lambda: qk_proj("k", wk_sb, bk_sb, kt_sb, 1, 0)],
                4: [lambda: qk_proj("k", wk_sb, bk_sb, kt_sb, 0, 2)],
                6: [lambda: qk_proj("k", wk_sb, bk_sb, kt_sb, 1, 1)],
                8: [lambda: qk_proj("k", wk_sb, bk_sb, kt_sb, 0, 3)],
                10: [lambda: qk_proj("q", wq_sb, bq_sb, qt_sb, 1, 0)],
                12: [lambda: qk_proj("k", wk_sb, bk_sb, kt_sb, 1, 2)],
                14: [lambda: qk_proj("k", wk_sb, bk_sb, kt_sb, 1, 3)],
                22: [lambda: qk_proj("q", wq_sb, bq_sb, qt_sb, 0, 1)],
                24: [lambda: qk_proj("q", wq_sb, bq_sb, qt_sb, 1, 1)],
                40: [lambda: qk_proj("q", wq_sb, bq_sb, qt_sb, 0, 2)],
                42: [lambda: qk_proj("q", wq_sb, bq_sb, qt_sb, 1, 2)],
                56: [lambda: qk_proj("q", wq_sb, bq_sb, qt_sb, 0, 3)],
                58: [lambda: qk_proj("q", wq_sb, bq_sb, qt_sb, 1, 3)],
            }
            # V projection units: one per key tile, from g=15
            vp_g = {}
            for kt in range(16):
                g = 15 + kt
                fillers.setdefault(g, []).append(
                    (lambda k: lambda: v_proj(k))(kt))
                vp_g[kt] = g

            # PV chunk queue with per-kt readiness
            pv_queue = [(b, kt) for b in range(8) for kt in range(16)]
            pv_pos = 0
            op_queue = []
            qt_done = set()

            def drain(g, budget):
                nonlocal pv_pos
                n = 0
                while pv_pos < len(pv_queue) and n < budget:
                    pb, pkt = pv_queue[pv_pos]
                    if g <= 16 * pb + pkt:        # u not produced yet
                        break
                    if g <= vp_g[pkt] + 1:        # vaug tile not ready
                        break
                    pv_pos += 1
                    pv_chunk(pb, pkt)
                    n += 1
                    if pkt == 15:
                        norm(pb)
                        pr, qt = BLOCKS[pb]
                        if pb % 2 == 1 and qt not in qt_done:
                            qt_done.add(qt)
                            op_queue.extend((qt, et) for et in range(NC))
                return n

            # ---- main loop ----
            for g in range(128):
                b, s = g // 16, g % 16
                scores_exp(b, s)
                fl = fillers.get(g, [])
                for f in fl:
                    f()
                backlog = (g - 16) - pv_pos
                if fl:
                    drain(g, 1)
                elif op_queue:
                    drain(g, 1 if backlog < 16 else 2)
                    qt, et = op_queue.pop(0)
                    out_proj(qt, et)
                else:
                    drain(g, 3 if backlog > 16 else 2)

            # ---- tail drain ----
            g = 128
            while pv_pos < len(pv_queue):
                drained = drain(g, 4)
                g += max(drained, 1)
            for qt, et in op_queue:
                out_proj(qt, et)

    nc.finalize()
    return nc


_NC_CACHE = None


def _get_nc():
    global _NC_CACHE
    if _NC_CACHE is None:
        _NC_CACHE = build_nc()
    return _NC_CACHE


def _swz(wT):
    """[C*128, cols] -> DMA-contiguous [128, C*cols] (partition-major)."""
    C = wT.shape[0] // 128
    return np.ascontiguousarray(
        wT.reshape(C, 128, -1).swapaxes(0, 1).reshape(128, -1)).astype(bf16)


def _xprep(x):
    """[T, D] -> [128, NT*NC*512]: tile tt gives [128(p), NC(c), 512(t)]
    where element (p, c, t) = x[tt*512 + t, c*128 + p]."""
    xT = np.asarray(x).T                      # [D, T]
    a = xT.reshape(NC, 128, NT, 512)          # [c, p, tt, t]
    a = a.transpose(1, 2, 0, 3)               # [p, tt, c, t]
    return np.ascontiguousarray(a.reshape(128, -1)).astype(bf16)


def _bcol(b, sl):
    return np.ascontiguousarray(
        np.asarray(b)[sl].reshape(2, 128).T).astype(np.float32)


def make_in_maps(query, key, value, wq, bq, wk, bk, wv, bv, wo, bo):
    xq_b = [_xprep(query[b]) for b in range(B)]
    xk_b = [_xprep(key[b]) for b in range(B)]
    xv_b = [_xprep(value[b]) for b in range(B)]
    in_maps = []
    for c in range(N_CORES):
        b, hg = divmod(c, HPC)
        sl = slice(hg * W, (hg + 1) * W)
        in_maps.append({
            "xq": xq_b[b],
            "xk": xk_b[b],
            "xv": xv_b[b],
            "wq": _swz(np.asarray(wq)[sl].T),
            "wk": _swz(np.asarray(wk)[sl].T),
            "wv": _swz(np.asarray(wv)[sl].T),
            "wo": _swz(np.asarray(wo)[:, sl].T),
            "bq": _bcol(bq, sl),
            "bk": _bcol(bk, sl),
            "bvb": np.ascontiguousarray(np.tile(
                np.asarray(bv)[sl].astype(np.float32)[None, :], (128, 1))),
        })
    return in_maps


def combine_outputs(outs, bo):
    full = np.zeros((B, T, D), np.float32)
    for c in range(N_CORES):
        b = c // HPC
        full[b] += outs[c].T
    full += np.asarray(bo, np.float32)[None, None, :]
    return full


def kernel(query, key, value, wq, bq, wk, bk, wv, bv, wo, bo):
    nc = _get_nc()
    in_maps = make_in_maps(query, key, value, wq, bq, wk, bk, wv, bv, wo, bo)
    res = run_bass_kernel_spmd(nc, in_maps, list(range(N_CORES)))
    outs = [np.asarray(res.results[c]["out"]) for c in range(N_CORES)]
    return combine_outputs(outs, bo)


# revision 10
# speedup vs baseline: 1.2292x; 1.0944x over previous
"""Multi-head attention (B=2, T=2048, D=1024, H=16) on 8 TRN2 NeuronCores.

Sharding: 2D (batch x head-group). Core c handles batch b = c // 4 and head
group hg = c % 4 (4 heads = 256 channels of the projected dim).

Single software-pipelined phase per core (no projection/attention barrier):
  - Warmup at t=0: dummy matmuls un-throttle the PE HAM clock gate and a
    dummy exp loads the ACT table set before real data arrives.
  - Inputs stream per 512-column t-tile ([128, 8, 512] staged) across BOTH
    DMA rings (HWDGE/sync + SWDGE/gpsimd); the critical first K/Q tiles are
    split in half across the rings so they land ~2x sooner. Q/K projections
    are split by j-half so only the jt=0 halves gate the first score
    matmuls (~12us in); ScalarE (the exp bottleneck, ~128us of ACTIVATE)
    then stays busy to the end.
  - V is projected directly into [t, j] layout (xv chunks stationary,
    N=256) - no transposes - and lands in the [V|1]-augmented PV weight
    tiles via one DVE add (bias broadcast from a host-replicated tile).
  - A static cost-aware scheduler walks 8 blocks x 16 key-tiles. Each step
    emits two row-concurrent score matmuls and one 1024-element exp, then
    fills the remaining PE budget (~1.1us/step) from queues: PV chunks
    (trailing exp; softmax denominator rides row 64 of the augmented
    weights), projection units (split into parts to bound per-step
    overshoot), normalization, output projection. O accumulators are
    copied to SBUF immediately after the last PV matmul so the two PSUM
    O banks recycle without waiting on the normalization chain.
  - Reciprocals use reciprocal_approx_fast (single custom-DVE op, run
    full-tile because the op mishandles single-row slices); 1/denom is
    broadcast over 64 partitions by a K=1 ones matmul. Output tiles DMA
    out on the sync ring (free after the input stream) as produced.

PSUM (8 banks): scores [128,2,512] x2 (4) + O accumulators [65,512] x2 (2)
+ aux ring [128,512] x2 (2, shared by projection / rb / out-proj tiles).

All shapes hardcoded. kernel() takes full inputs, returns [2, 2048, 1024].
"""

import numpy as np
import ml_dtypes

import concourse.bass as bass
import concourse.bacc as bacc
import concourse.mybir as mybir
import concourse.tile as tile
from concourse.bass_utils import run_bass_kernel_spmd

B, T, D, H, Hd = 2, 2048, 1024, 16, 64
HPC = 4          # heads per core
W = HPC * Hd     # 256 projected channels per core
SCALE = Hd ** -0.5
N_CORES = 8
NT = 4           # 512-wide t-tiles
NC = 8           # 128-deep contraction chunks

BF16 = mybir.dt.bfloat16
F32 = mybir.dt.float32
bf16 = ml_dtypes.bfloat16

BLOCKS = [(0, 0), (1, 0), (0, 1), (1, 1), (0, 2), (1, 2), (0, 3), (1, 3)]

# PE cost model (ns) for the step scheduler
C_SCORE, C_PV, C_OP, C_NORM = 280, 440, 520, 450
C_KQ_PART, C_VP_PART = 900, 700
STEP_CAP = 1090


def build_nc():
    nc = bacc.Bacc("TRN2", target_bir_lowering=False, debug=False)

    xq = nc.dram_tensor("xq", [128, NT * NC * 512], BF16, kind="ExternalInput").ap()
    xk = nc.dram_tensor("xk", [128, NT * NC * 512], BF16, kind="ExternalInput").ap()
    xv = nc.dram_tensor("xv", [128, NT * NC * 512], BF16, kind="ExternalInput").ap()
    wq = nc.dram_tensor("wq", [128, NC * W], BF16, kind="ExternalInput").ap()
    wk = nc.dram_tensor("wk", [128, NC * W], BF16, kind="ExternalInput").ap()
    wv = nc.dram_tensor("wv", [128, NC * W], BF16, kind="ExternalInput").ap()
    wo = nc.dram_tensor("wo", [128, 2 * D], BF16, kind="ExternalInput").ap()
    bq = nc.dram_tensor("bq", [128, 2], F32, kind="ExternalInput").ap()
    bk = nc.dram_tensor("bk", [128, 2], F32, kind="ExternalInput").ap()
    bvb = nc.dram_tensor("bvb", [128, 256], F32, kind="ExternalInput").ap()
    out = nc.dram_tensor("out", [D, T], F32, kind="ExternalOutput").ap()

    xq_t = xq.rearrange("p (t c q) -> p t c q", c=NC, q=512)
    xk_t = xk.rearrange("p (t c q) -> p t c q", c=NC, q=512)
    xv_t = xv.rearrange("p (t c q) -> p t c q", c=NC, q=512)

    Exp = mybir.ActivationFunctionType.Exp
    Add = mybir.AluOpType.add

    with tile.TileContext(nc) as tc:
        with (
            tc.tile_pool(name="persist", bufs=1) as persist,
            tc.tile_pool(name="xst", bufs=5) as xst,
            tc.tile_pool(name="upool", bufs=44) as upool,
            tc.tile_pool(name="small", bufs=2) as small,
            tc.tile_pool(name="stgp", bufs=4) as stgp,
            tc.tile_pool(name="ps", bufs=1, space="PSUM") as ps,
        ):
            # ---- warmup: HAM un-throttle + ACT table load ----
            bcast1 = persist.tile([65, 64], BF16, tag="bcast1")
            nc.vector.memset(bcast1, 1.0)
            wdum = persist.tile([64, 512], BF16, tag="wdum")
            nc.vector.memset(wdum, 1.0)
            nc.scalar.activation(wdum[:, 0:64], bcast1[0:64, :], Exp,
                                 scale=0.1)
            wps = ps.tile([64, 512], F32, tag="aux", bufs=2, name="aux")
            for i in range(10):
                nc.tensor.matmul(wps, lhsT=bcast1[0:64, :], rhs=wdum,
                                 start=True, stop=True)

            # ---- persistent weights / activations ----
            wk_sb = persist.tile([128, NC, W], BF16, tag="wk")
            wq_sb = persist.tile([128, NC, W], BF16, tag="wq")
            wv_sb = persist.tile([128, NC, W], BF16, tag="wv")
            wo_sb = persist.tile([128, 2, D], BF16, tag="wo")
            bq_sb = persist.tile([128, 2], F32, tag="bq")
            bk_sb = persist.tile([128, 2], F32, tag="bk")
            bvb_sb = persist.tile([128, 256], F32, tag="bvb")

            qt_sb = persist.tile([128, 2, T], BF16, tag="qt")   # Q.T [j, t]
            kt_sb = persist.tile([128, 2, T], BF16, tag="kt")   # K.T [j, t]
            otn_sb = persist.tile([128, 2, T], BF16, tag="otn")  # normalized O.T
            # V augmented with ones column per head: [k, kt16, h4, 0:64]=V
            vaug = persist.tile([128, 16, HPC, Hd + 1], BF16, tag="vaug")
            nc.vector.memset(vaug[:, :, :, 64:65], 1.0)

            # ---- DMA issue: both rings, priority order ----
            x_tiles = {}

            def dma_x(name, dram, tt, eng):
                t = xst.tile([128, NC, 512], BF16, tag="xst", name="xst")
                eng.dma_start(out=t, in_=dram[:, tt])
                x_tiles[(name, tt)] = t

            def dma_x_split(name, dram, tt):
                # halves across both rings so the tile lands ~2x sooner
                t = xst.tile([128, NC, 512], BF16, tag="xst", name="xst")
                nc.sync.dma_start(out=t[:, :, 0:256], in_=dram[:, tt, :, 0:256])
                nc.gpsimd.dma_start(out=t[:, :, 256:512],
                                    in_=dram[:, tt, :, 256:512])
                x_tiles[(name, tt)] = t

            dma_x_split("k", xk_t, 0)
            nc.sync.dma_start(out=wk_sb, in_=wk.rearrange("p (c j) -> p c j", j=W))
            nc.gpsimd.dma_start(out=wq_sb, in_=wq.rearrange("p (c j) -> p c j", j=W))
            dma_x_split("q", xq_t, 0)
            nc.sync.dma_start(out=bk_sb, in_=bk)
            nc.sync.dma_start(out=bq_sb, in_=bq)
            dma_x("k", xk_t, 1, nc.sync)
            dma_x("k", xk_t, 2, nc.gpsimd)
            dma_x("k", xk_t, 3, nc.sync)
            nc.gpsimd.dma_start(out=wv_sb, in_=wv.rearrange("p (c j) -> p c j", j=W))
            nc.gpsimd.dma_start(out=bvb_sb, in_=bvb)
            dma_x("v", xv_t, 0, nc.gpsimd)
            dma_x("v", xv_t, 1, nc.sync)
            dma_x("v", xv_t, 2, nc.gpsimd)
            dma_x("q", xq_t, 1, nc.sync)
            dma_x("v", xv_t, 3, nc.gpsimd)
            nc.sync.dma_start(out=wo_sb, in_=wo.rearrange("p (c e) -> p c e", e=D))
            dma_x("q", xq_t, 2, nc.gpsimd)
            dma_x("q", xq_t, 3, nc.sync)

            # ---- projection units (as multi-part work items) ----
            aux_hold = {}

            def kq_part(name, w_sb, b_sb, dst, jt, tt, half):
                xc = x_tiles[(name, tt)]
                key = (name, jt, tt)
                if half == 0:
                    aux_hold[key] = ps.tile([128, 512], F32, tag="aux",
                                            bufs=2, name="aux")
                p = aux_hold[key]
                for c in range(4 * half, 4 * half + 4):
                    nc.tensor.matmul(
                        p, lhsT=w_sb[:, c, jt * 128:(jt + 1) * 128],
                        rhs=xc[:, c, :], start=(c == 0), stop=(c == NC - 1))
                if half == 1:
                    nc.vector.tensor_scalar_add(
                        dst[:, jt, tt * 512:(tt + 1) * 512], p,
                        b_sb[:, jt:jt + 1])
                    del aux_hold[key]

            def vp_part(kt, half):
                xc = x_tiles[("v", kt // 4)]
                tl = kt % 4
                key = ("vp", kt)
                if half == 0:
                    aux_hold[key] = ps.tile([128, 256], F32, tag="aux",
                                            bufs=2, name="aux")
                p = aux_hold[key]
                for c in range(4 * half, 4 * half + 4):
                    nc.tensor.matmul(
                        p, lhsT=xc[:, c, tl * 128:(tl + 1) * 128],
                        rhs=wv_sb[:, c, :], start=(c == 0), stop=(c == NC - 1))
                if half == 1:
                    nc.vector.tensor_tensor(
                        out=vaug[:, kt, :, 0:64],
                        in0=p.rearrange("p (h d) -> p h d", h=HPC),
                        in1=bvb_sb.rearrange("p (h d) -> p h d", h=HPC),
                        op=Add)
                    del aux_hold[key]

            def kq_unit(name, w_sb, b_sb, dst, jt, tt):
                return [
                    (C_KQ_PART,
                     lambda h=h: kq_part(name, w_sb, b_sb, dst, jt, tt, h))
                    for h in range(2)]

            def vp_unit(kt):
                return [(C_VP_PART, lambda h=h: vp_part(kt, h))
                        for h in range(2)]

            # ---- attention pieces ----
            us = {}
            cur_o = {}

            def scores_exp(b, kt):
                pr, qt = BLOCKS[b]
                qsl = slice(qt * 512, (qt + 1) * 512)
                s = ps.tile([128, 2, 512], F32, tag="s", bufs=2, name="s")
                for m in range(2):
                    po = 64 * m
                    nc.tensor.matmul(
                        s[:, m, :],
                        lhsT=kt_sb[po:po + 64, pr, kt * 128:(kt + 1) * 128],
                        rhs=qt_sb[po:po + 64, pr, qsl],
                        start=True, stop=True)
                u = upool.tile([128, 2, 512], BF16, tag="u", name="u")
                nc.scalar.activation(u, s, Exp, scale=SCALE)
                us[(b, kt)] = u

            def pv_chunk(b, kt):
                pr, qt = BLOCKS[b]
                if kt == 0:
                    cur_o[b] = (
                        ps.tile([65, 512], F32, tag="oA", bufs=1, name="oA"),
                        ps.tile([65, 512], F32, tag="oB", bufs=1, name="oB"))
                oa, ob = cur_o[b]
                u = us.pop((b, kt))
                for m, o_ps in ((0, oa), (1, ob)):
                    nc.tensor.matmul(
                        o_ps, lhsT=vaug[:, kt, 2 * pr + m, :],
                        rhs=u[:, m, :],
                        start=(kt == 0), stop=(kt == 15))
                if kt == 15:
                    # free the O PSUM banks early: norm works off SBUF copies
                    raws = []
                    for m, o_ps in ((0, oa), (1, ob)):
                        raw = small.tile([65, 512], F32, tag="oraw", bufs=4,
                                         name="oraw")
                        nc.vector.tensor_copy(raw, o_ps)
                        raws.append(raw)
                    cur_o[b] = raws

            def norm(b):
                pr, qt = BLOCKS[b]
                qsl = slice(qt * 512, (qt + 1) * 512)
                for m, raw in ((0, cur_o[b][0]), (1, cur_o[b][1])):
                    # approx_fast breaks on single-row slices; run full-tile
                    # (same per-lane cost), only row 64 (denom) is used.
                    rf = small.tile([65, 512], F32, tag="rf", name="rf")
                    nc.vector.reciprocal_approx_fast(out=rf, in_=raw)
                    r16 = small.tile([65, 512], BF16, tag="r16", name="r16")
                    nc.vector.tensor_copy(r16[64:65, :], rf[64:65, :])
                    rbp = ps.tile([64, 512], F32, tag="aux", bufs=2, name="aux")
                    nc.tensor.matmul(rbp, lhsT=bcast1[64:65, :],
                                     rhs=r16[64:65, :], start=True, stop=True)
                    rbs = small.tile([64, 512], F32, tag="rbs", name="rbs")
                    nc.vector.tensor_copy(rbs, rbp)
                    if m == 0:
                        nc.vector.tensor_mul(
                            otn_sb[0:64, pr, qsl], raw[0:64, :], rbs)
                    else:
                        otnB = small.tile([64, 512], BF16, tag="otnB",
                                          name="otnB")
                        nc.vector.tensor_mul(otnB, raw[0:64, :], rbs)
                        nc.gpsimd.dma_start(out=otn_sb[64:128, pr, qsl],
                                            in_=otnB)

            def out_proj(qt, et):
                qsl = slice(qt * 512, (qt + 1) * 512)
                e = ps.tile([128, 512], F32, tag="aux", bufs=2, name="aux")
                for jc in range(2):
                    nc.tensor.matmul(
                        e, lhsT=wo_sb[:, jc, et * 128:(et + 1) * 128],
                        rhs=otn_sb[:, jc, qsl],
                        start=(jc == 0), stop=(jc == 1))
                stg = stgp.tile([128, 512], F32, tag="stg", name="stg")
                nc.vector.tensor_copy(stg, e)
                nc.sync.dma_start(out=out[et * 128:(et + 1) * 128, qsl],
                                  in_=stg)

            # ---- prologue projections (jt=0, t=0 only) ----
            kq_part("k", wk_sb, bk_sb, kt_sb, 0, 0, 0)
            kq_part("k", wk_sb, bk_sb, kt_sb, 0, 0, 1)
            kq_part("q", wq_sb, bq_sb, qt_sb, 0, 0, 0)
            kq_part("q", wq_sb, bq_sb, qt_sb, 0, 0, 1)

            # ---- filler work queue: (deadline, earliest, parts) ----
            work = []
            work.append((4, 0, kq_unit("k", wk_sb, bk_sb, kt_sb, 0, 1)))
            work.append((8, 1, kq_unit("k", wk_sb, bk_sb, kt_sb, 0, 2)))
            work.append((12, 2, kq_unit("k", wk_sb, bk_sb, kt_sb, 0, 3)))
            work.append((14, 0, kq_unit("q", wq_sb, bq_sb, qt_sb, 1, 0)))
            work.append((15, 1, kq_unit("k", wk_sb, bk_sb, kt_sb, 1, 0)))
            work.append((19, 2, kq_unit("k", wk_sb, bk_sb, kt_sb, 1, 1)))
            work.append((23, 2, kq_unit("k", wk_sb, bk_sb, kt_sb, 1, 2)))
            work.append((27, 3, kq_unit("k", wk_sb, bk_sb, kt_sb, 1, 3)))
            for kt in range(16):
                work.append((18 + kt, 10 + 2 * (kt // 4), vp_unit(kt)))
            work.append((30, 9, kq_unit("q", wq_sb, bq_sb, qt_sb, 0, 1)))
            work.append((31, 9, kq_unit("q", wq_sb, bq_sb, qt_sb, 1, 1)))
            work.append((62, 12, kq_unit("q", wq_sb, bq_sb, qt_sb, 0, 2)))
            work.append((63, 12, kq_unit("q", wq_sb, bq_sb, qt_sb, 1, 2)))
            work.append((94, 12, kq_unit("q", wq_sb, bq_sb, qt_sb, 0, 3)))
            work.append((95, 12, kq_unit("q", wq_sb, bq_sb, qt_sb, 1, 3)))
            work.sort(key=lambda w: w[0])
            widx = 0
            inprog = None     # parts list of the started unit

            # PV queue state
            pv_queue = [(b, kt) for b in range(8) for kt in range(16)]
            pv_pos = 0
            vp_emit_g = {}    # kt -> step when vp unit fully emitted
            pv15_g = {}       # b -> step when chunk 15 emitted
            op_queue = []
            qt_done = set()

            def pv_ready(g):
                if pv_pos >= len(pv_queue):
                    return False
                pb, pkt = pv_queue[pv_pos]
                if g <= 16 * pb + pkt:
                    return False
                if vp_emit_g.get(pkt) is None or g <= vp_emit_g[pkt]:
                    return False
                if pkt == 0 and pb > 0 and g < pv15_g.get(pb - 1, -9) + 2:
                    return False
                return True

            def emit_pv(g):
                nonlocal pv_pos
                pb, pkt = pv_queue[pv_pos]
                pv_pos += 1
                pv_chunk(pb, pkt)
                if pkt == 15:
                    pv15_g[pb] = g
                    norm(pb)
                    pr, qt = BLOCKS[pb]
                    if pb % 2 == 1 and qt not in qt_done:
                        qt_done.add(qt)
                        op_queue.extend((qt, et) for et in range(NC))
                    return C_PV + C_NORM
                return C_PV

            def run_step(g, cap):
                nonlocal widx, inprog
                spent = 0
                while spent < cap:
                    lag = (g - 4) - pv_pos
                    # finish a started multi-part unit first
                    if inprog:
                        cost, fn = inprog.pop(0)
                        if not inprog:
                            inprog = None
                        fn()
                        spent += cost
                        continue
                    # urgent pv (u-ring pressure)
                    if lag > 22 and pv_ready(g):
                        spent += emit_pv(g)
                        continue
                    # overdue filler
                    if widx < len(work) and work[widx][0] <= g + 2 \
                            and work[widx][1] <= g:
                        _, _, parts = work[widx]
                        widx += 1
                        inprog = list(parts)
                        cost, fn = inprog.pop(0)
                        if not inprog:
                            inprog = None
                        fn()
                        spent += cost
                        continue
                    # steady pv
                    if pv_ready(g) and lag > 4:
                        spent += emit_pv(g)
                        continue
                    # non-urgent filler if it fits
                    if widx < len(work) and work[widx][1] <= g \
                            and spent + work[widx][2][0][0] <= cap:
                        _, _, parts = work[widx]
                        widx += 1
                        inprog = list(parts)
                        cost, fn = inprog.pop(0)
                        if not inprog:
                            inprog = None
                        fn()
                        spent += cost
                        continue
                    # out-proj
                    if op_queue and spent + C_OP <= cap + 200:
                        qt, et = op_queue.pop(0)
                        out_proj(qt, et)
                        spent += C_OP
                        continue
                    # trailing pv
                    if pv_ready(g):
                        spent += emit_pv(g)
                        continue
                    break
                return spent

            # record the step at which each vp unit finishes emitting
            # (vp_unit closures resolve `vp_part` at call time)
            cur_g = [0]
            _orig_vp_part = vp_part

            def vp_part_mark(kt, half):
                _orig_vp_part(kt, half)
                if half == 1:
                    vp_emit_g[kt] = cur_g[0]
            vp_part = vp_part_mark  # noqa: F811

            # ---- main loop ----
            for g in range(128):
                cur_g[0] = g
                b, s = g // 16, g % 16
                scores_exp(b, s)
                run_step(g, STEP_CAP - C_SCORE)

            # ---- tail drain ----
            g = 128
            while pv_pos < len(pv_queue) or inprog or widx < len(work):
                cur_g[0] = g
                spent = run_step(g, 4000)
                g += 1
                if spent == 0:
                    g += 1  # safety: advance readiness horizon
            for qt, et in op_queue:
                out_proj(qt, et)

    nc.finalize()
    return nc


_NC_CACHE = None


def _get_nc():
    global _NC_CACHE
    if _NC_CACHE is None:
        _NC_CACHE = build_nc()
    return _NC_CACHE


def _swz(wT):
    """[C*128, cols] -> DMA-contiguous [128, C*cols] (partition-major)."""
    C = wT.shape[0] // 128
    return np.ascontiguousarray(
        wT.reshape(C, 128, -1).swapaxes(0, 1).reshape(128, -1)).astype(bf16)


def _xprep(x):
    """[T, D] -> [128, NT*NC*512]: tile tt gives [128(p), NC(c), 512(t)]
    where element (p, c, t) = x[tt*512 + t, c*128 + p]."""
    xT = np.asarray(x).T                      # [D, T]
    a = xT.reshape(NC, 128, NT, 512)          # [c, p, tt, t]
    a = a.transpose(1, 2, 0, 3)               # [p, tt, c, t]
    return np.ascontiguousarray(a.reshape(128, -1)).astype(bf16)


def _bcol(b, sl):
    return np.ascontiguousarray(
        np.asarray(b)[sl].reshape(2, 128).T).astype(np.float32)


def make_in_maps(query, key, value, wq, bq, wk, bk, wv, bv, wo, bo):
    xq_b = [_xprep(query[b]) for b in range(B)]
    xk_b = [_xprep(key[b]) for b in range(B)]
    xv_b = [_xprep(value[b]) for b in range(B)]
    in_maps = []
    for c in range(N_CORES):
        b, hg = divmod(c, HPC)
        sl = slice(hg * W, (hg + 1) * W)
        in_maps.append({
            "xq": xq_b[b],
            "xk": xk_b[b],
            "xv": xv_b[b],
            "wq": _swz(np.asarray(wq)[sl].T),
            "wk": _swz(np.asarray(wk)[sl].T),
            "wv": _swz(np.asarray(wv)[sl].T),
            "wo": _swz(np.asarray(wo)[:, sl].T),
            "bq": _bcol(bq, sl),
            "bk": _bcol(bk, sl),
            "bvb": np.ascontiguousarray(np.tile(
                np.asarray(bv)[sl].astype(np.float32)[None, :], (128, 1))),
        })
    return in_maps


def combine_outputs(outs, bo):
    full = np.zeros((B, T, D), np.float32)
    for c in range(N_CORES):
        b = c // HPC
        full[b] += outs[c].T
    full += np.asarray(bo, np.float32)[None, None, :]
    return full


def kernel(query, key, value, wq, bq, wk, bk, wv, bv, wo, bo):
    nc = _get_nc()
    in_maps = make_in_maps(query, key, value, wq, bq, wk, bk, wv, bv, wo, bo)
    res = run_bass_kernel_spmd(nc, in_maps, list(range(N_CORES)))
    outs = [np.asarray(res.results[c]["out"]) for c in range(N_CORES)]
    return combine_outputs(outs, bo)
